# revision 1
# baseline (speedup 1.0000x reference)
"""3-layer GCN encoder on 8 Trainium2 NeuronCores.

Strategy:
- Nodes sharded across 8 cores (6250 real nodes each, padded to 6272 = 49*128
  slots); edges partitioned by destination core.
- GCN norm is symmetric (norm = dinv[src]*dinv[dst]), so node rows in the
  gather table are pre-scaled by dinv; aggregation is a plain sum of gathered
  rows; the result is post-scaled by dinv[dst].
- Aggregate-then-transform: A_hat @ (h W) == (A_hat @ h) W, so every gather
  moves 64-dim (256B) rows regardless of layer.
- Scatter-add is eliminated on-device: each core's destination nodes are
  degree-sorted into 128-node blocks; every node's edge list is padded to the
  block max degree k_b (pad slots point at an all-zero table row). Gather
  indices are laid out slot-major so edge j of block-node p lands at
  (partition p, chunk j) — segment-sum becomes a fixed-shape strided
  tensor_reduce per block.
- dma_gather indices are signed int16, so the 50184-row table is addressed
  through two windows: L = rows of cores 0-4 (31365 rows), H = rows of cores
  5-7 (18819 rows). L and H edge sets get independent degree-sorted layouts;
  the H partial accumulator is realigned to L order via a small 6272-row
  gather through a DRAM scratch buffer.
- Inter-layer exchange: each core AllGathers its 6273-row shard (6272 outputs
  + 1 zero row) into the next layer's replicated table.
"""

import numpy as np


def _install_ntff_hook_shim():
    """The axon boot registers its NTFF profile hook via
    ``antenv.axon_hooks`` — a module this image's antenv package lacks.
    Pre-seed an equivalent holder module so trace=True can profile.
    Must run before jax initializes the axon platform."""
    import sys
    import types

    if "antenv.axon_hooks" in sys.modules:
        return
    mod = types.ModuleType("antenv.axon_hooks")
    holder = [None]
    mod.set_axon_ntff_profile_hook = lambda h: holder.__setitem__(0, h)
    mod.get_axon_ntff_profile_hook = lambda: holder[0]
    sys.modules["antenv.axon_hooks"] = mod
    try:
        import antenv

        antenv.axon_hooks = mod
    except ImportError:
        pass


_install_ntff_hook_shim()

N = 50000
E = 800000
D = 64
DOUT = 32
C = 8
NPC = 6250            # real nodes per core
SLOTS = 6272          # padded slots per core = 49*128
B = 49                # dst blocks per core
SHARD = 6273          # table rows per core (slots + 1 zero row)
L_CORES = 5
L_WIN = L_CORES * SHARD          # 31365
H_WIN = (C - L_CORES) * SHARD    # 18819
TROWS = C * SHARD                # 50184
ZROW = SLOTS                     # zero-row offset within each shard
PIECE_CAP = 8192                 # rows per gather buffer (whole blocks)
GCALL = 1024                     # max rows per dma_gather call (SWDGE
                                 # descriptor-carveout limit: 1024 descs)

_last_results = None  # BassKernelResults of the most recent run (for test.py)


def _wrap_idx(stream):
    """int32 stream -> int16 [128, len/16] wrapped-and-replicated index tile."""
    n = stream.shape[-1]
    assert n % 16 == 0
    w = stream.reshape(-1, n // 16, 16)            # [C?, n/16, 16]
    w = np.swapaxes(w, -1, -2)                     # [..., 16, n/16]
    w = np.tile(w, (1, 8, 1)) if w.ndim == 3 else np.tile(w, (8, 1))
    return np.ascontiguousarray(w).astype(np.int16)


def _slot_layout(dloc, q, kb):
    """Build per-core slot-major index streams.

    dloc: [C, NPC] per-node local degree (in this window), in node-id order
    q:    [C, NPC] per-node position in this window's sorted order
    kb:   [B] shared block slot counts
    Returns (off, positions builder helpers) — used by caller.
    """
    off = np.zeros(B + 1, np.int64)
    off[1:] = np.cumsum(kb)
    return off


def _prep(x, edge_index, W1, b1, W2, b2, W3, b3):
    src = np.asarray(edge_index[0], dtype=np.int64)
    dst = np.asarray(edge_index[1], dtype=np.int64)
    loop = np.arange(N, dtype=np.int64)
    src = np.concatenate([src, loop])
    dst = np.concatenate([dst, loop])

    deg = np.bincount(dst, minlength=N)
    dinv = np.zeros(N, np.float64)
    nz = deg > 0
    dinv[nz] = 1.0 / np.sqrt(deg[nz].astype(np.float64))
    dinv = dinv.astype(np.float32)

    src_core = src // NPC
    is_L = src_core < L_CORES
    d0 = np.bincount(dst[is_L], minlength=N)
    d1 = deg - d0

    # Per-core sorted orders. q0[n] = position of node n in its core's
    # L-order (also its table-row offset); q1[n] = position in H-order.
    q0 = np.empty(N, np.int64)
    q1 = np.empty(N, np.int64)
    k0 = np.zeros(B, np.int64)
    k1 = np.zeros(B, np.int64)
    for c in range(C):
        nodes = np.arange(c * NPC, (c + 1) * NPC)
        o0 = nodes[np.argsort(d0[nodes], kind="stable")]
        o1 = nodes[np.argsort(d1[nodes], kind="stable")]
        q0[o0] = np.arange(NPC)
        q1[o1] = np.arange(NPC)
        p0 = np.zeros(SLOTS, np.int64)
        p0[:NPC] = d0[o0]
        p1 = np.zeros(SLOTS, np.int64)
        p1[:NPC] = d1[o1]
        k0 = np.maximum(k0, p0.reshape(B, 128).max(axis=1))
        k1 = np.maximum(k1, p1.reshape(B, 128).max(axis=1))
    k0 = k0.astype(int)
    k1 = k1.astype(int)
    off0 = np.zeros(B + 1, np.int64)
    off0[1:] = np.cumsum(k0)
    off1 = np.zeros(B + 1, np.int64)
    off1[1:] = np.cumsum(k1)
    L_len = int(128 * off0[-1])
    H_len = int(128 * off1[-1])

    trow = (np.arange(N) // NPC) * SHARD + q0  # table row of each node

    # --- index streams -------------------------------------------------
    def build_streams(sel, qx, offx, length, base):
        """sel: edge mask for this window; qx: dst position array;
        offx: block offsets; base: subtracted from src table row."""
        es, ed = src[sel], dst[sel]
        core = ed // NPC
        qd = qx[ed]
        order = np.argsort(core * SLOTS + qd, kind="stable")
        es, ed, core, qd = es[order], ed[order], core[order], qd[order]
        # rank j of each edge within its destination's list
        key = core * SLOTS + qd
        starts = np.searchsorted(key, key)  # first occurrence index per key
        j = np.arange(len(key)) - starts
        b = qd // 128
        p = qd % 128
        pos = (offx[b] + j) * 128 + p
        streams = np.full((C, length), ZROW, np.int32)
        streams[core, pos] = (trow[es] - base).astype(np.int32)
        return streams

    sL = build_streams(is_L, q0, off0, L_len, 0)
    sH = build_streams(~is_L, q1, off1, H_len, L_WIN)

    idxL = _wrap_idx(sL)            # [C, 128, L_len/16] int16
    idxH = _wrap_idx(sH)

    # --- realign: for L-position i, the scratch row (p1*B + b1) ---------
    idxR = np.empty((C, SLOTS), np.int32)
    for c in range(C):
        nodes = np.arange(c * NPC, (c + 1) * NPC)
        r = np.full(SLOTS, 0, np.int32)
        qq0 = q0[nodes]
        qq1 = q1[nodes]
        r[qq0] = ((qq1 % 128) * B + qq1 // 128).astype(np.int32)
        dummy = np.arange(NPC, SLOTS)
        r[dummy] = ((dummy % 128) * B + dummy // 128).astype(np.int32)
        idxR[c] = r
    idxRw = _wrap_idx(idxR)

    # --- dinv tiles [C, 128, B]: value at (p, b) = dinv(node at q0=b*128+p)
    dinv_t = np.zeros((C, 128, B), np.float32)
    for c in range(C):
        nodes = np.arange(c * NPC, (c + 1) * NPC)
        arr = np.zeros(SLOTS, np.float32)
        arr[q0[nodes]] = dinv[nodes]
        dinv_t[c] = arr.reshape(B, 128).T

    # --- initial table: prescaled, permuted x ---------------------------
    x = np.asarray(x, np.float32)
    xt = np.zeros((TROWS, D), np.float32)
    xt[trow] = x * dinv[:, None]

    meta = dict(k0=k0, k1=k1, off0=off0, off1=off1, L_len=L_len, H_len=H_len)
    host = dict(
        x_table=xt,
        idxL=idxL, idxH=idxH, idxR=idxRw, dinv_t=dinv_t,
        W1=np.asarray(W1, np.float32), W2=np.asarray(W2, np.float32),
        W3=np.asarray(W3, np.float32),
        b1=np.asarray(b1, np.float32).reshape(D, 1),
        b2=np.asarray(b2, np.float32).reshape(D, 1),
        b3=np.asarray(b3, np.float32).reshape(DOUT, 1),
        q0=q0, trow=trow,
    )
    return meta, host


def _pieces(kb, off):
    """Group blocks into pieces with <= PIECE_CAP gathered rows each.
    Returns list of (b_start, b_end, row_off, rows)."""
    out = []
    bs = 0
    while bs < B:
        be = bs
        rows = 0
        while be < B and (rows + 128 * kb[be]) <= PIECE_CAP:
            rows += 128 * kb[be]
            be += 1
        if be == bs:  # single oversized block
            rows = 128 * kb[bs]
            be = bs + 1
        out.append((bs, be, int(128 * off[bs]), int(rows)))
        bs = be
    return out


def _build(meta):
    import concourse.bacc as bacc
    import concourse.mybir as mybir
    from concourse.tile import TileContext

    k0, k1 = meta["k0"], meta["k1"]
    off0, off1 = meta["off0"], meta["off1"]
    L_len, H_len = meta["L_len"], meta["H_len"]
    f32 = mybir.dt.float32
    i16 = mybir.dt.int16

    nc = bacc.Bacc(None, target_bir_lowering=False, num_swdge_queues=4)

    x_table = nc.declare_dram_parameter("x_table", [TROWS, D], f32, isOutput=False)
    idxL_p = nc.declare_dram_parameter("idxL", [128, L_len // 16], i16, isOutput=False)
    idxH_p = nc.declare_dram_parameter("idxH", [128, H_len // 16], i16, isOutput=False)
    idxR_p = nc.declare_dram_parameter("idxR", [128, SLOTS // 16], i16, isOutput=False)
    dinv_p = nc.declare_dram_parameter("dinv_t", [128, B], f32, isOutput=False)
    W_p = [nc.declare_dram_parameter(f"W{i+1}", [D, D if i < 2 else DOUT], f32, isOutput=False) for i in range(3)]
    b_p = [nc.declare_dram_parameter(f"b{i+1}", [D if i < 2 else DOUT, 1], f32, isOutput=False) for i in range(3)]
    z_ext = nc.declare_dram_parameter("z", [SLOTS, DOUT], f32, isOutput=True)

    tables = [x_table]
    agins = []
    scratches = []
    for l in range(2):
        tables.append(nc.dram_tensor(f"table{l+1}", [TROWS, D], f32, addr_space="Shared"))
        agins.append(nc.dram_tensor(f"agin{l}", [SHARD, D], f32))
    for l in range(3):
        scratches.append(nc.dram_tensor(f"scratch{l}", [SLOTS, D], f32))

    piecesL = _pieces(k0, off0)
    piecesH = _pieces(k1, off1)
    qctr = [0]

    def next_q():
        q = qctr[0] % 4
        qctr[0] += 1
        return q

    with TileContext(nc) as tc:
        with (
            tc.tile_pool(name="const", bufs=1) as cpool,
            tc.tile_pool(name="acc", bufs=1) as apool,
            tc.tile_pool(name="gath", bufs=2) as gpool,
            tc.tile_pool(name="stage", bufs=4) as spool,
            tc.tile_pool(name="psum", bufs=2, space="PSUM") as ppool,
        ):
            # ---- persistent constants ----
            idxL_t = cpool.tile([128, L_len // 16], i16, tag="idxL")
            idxH_t = cpool.tile([128, H_len // 16], i16, tag="idxH")
            idxR_t = cpool.tile([128, SLOTS // 16], i16, tag="idxR")
            dinv_t = cpool.tile([128, B], f32, tag="dinv")
            ident = cpool.tile([128, 128], f32, tag="ident")
            zrow = cpool.tile([1, D], f32, tag="zrow")
            Wt = [cpool.tile([D, D if i < 2 else DOUT], f32, tag=f"W{i}", name=f"Wt{i}") for i in range(3)]
            bt = [cpool.tile([D if i < 2 else DOUT, 1], f32, tag=f"b{i}", name=f"bt{i}") for i in range(3)]

            nc.sync.dma_start(out=idxL_t[:], in_=idxL_p[:])
            nc.sync.dma_start(out=idxH_t[:], in_=idxH_p[:])
            nc.sync.dma_start(out=idxR_t[:], in_=idxR_p[:])
            nc.sync.dma_start(out=dinv_t[:], in_=dinv_p[:])
            for i in range(3):
                nc.sync.dma_start(out=Wt[i][:], in_=W_p[i][:])
                nc.sync.dma_start(out=bt[i][:], in_=b_p[i][:])
            nc.gpsimd.memset(ident[:], 1.0)
            nc.gpsimd.affine_select(
                out=ident[:], in_=ident[:], pattern=[[-1, 128]], base=0,
                channel_multiplier=1, compare_op=mybir.AluOpType.is_equal, fill=0.0)
            nc.vector.memset(zrow[:], 0.0)
            for l in range(2):
                nc.sync.dma_start(out=agins[l][ZROW:ZROW + 1, :], in_=zrow[:])

            # ---- layers ----
            for l in range(3):
                table = tables[l]
                Dl = D if l < 2 else DOUT
                acc0 = apool.tile([128, B, D], f32, tag="acc0")
                acc1 = apool.tile([128, B, D], f32, tag="acc1")
                accR = apool.tile([128, B, D], f32, tag="accR")

                # H phase first (feeds scratch -> realign gather)
                for (bs, be, roff, rows) in piecesH:
                    gh = gpool.tile([128, rows // 128, D], f32, tag="gh")
                    for s0 in range(0, rows, GCALL):
                        sn = min(GCALL, rows - s0)
                        nc.gpsimd.dma_gather(
                            out_ap=gh[:, s0 // 128:(s0 + sn) // 128, :],
                            in_ap=table[L_WIN:TROWS, :],
                            idxs_ap=idxH_t[:, (roff + s0) // 16:(roff + s0 + sn) // 16],
                            num_idxs=sn, num_idxs_reg=sn, elem_size=D,
                            queue_num=next_q())
                    for b in range(bs, be):
                        o = int(128 * (off1[b] - off1[bs])) // 128
                        kb = int(k1[b])
                        nc.vector.tensor_reduce(
                            out=acc1[:, b, :],
                            in_=gh[:, o:o + kb, :].rearrange("p k d -> p d k"),
                            axis=mybir.AxisListType.X, op=mybir.AluOpType.add)
                # acc1 -> scratch (p-major mirror), then realign gather
                nc.sync.dma_start(
                    out=scratches[l][:].rearrange("(p b) d -> p b d", p=128),
                    in_=acc1[:])
                for s0 in range(0, SLOTS, GCALL):
                    sn = min(GCALL, SLOTS - s0)
                    nc.gpsimd.dma_gather(
                        out_ap=accR[:, s0 // 128:(s0 + sn) // 128, :],
                        in_ap=scratches[l][:],
                        idxs_ap=idxR_t[:, s0 // 16:(s0 + sn) // 16],
                        num_idxs=sn, num_idxs_reg=sn, elem_size=D,
                        queue_num=next_q())

                # L phase
                for (bs, be, roff, rows) in piecesL:
                    gl = gpool.tile([128, rows // 128, D], f32, tag="gl")
                    for s0 in range(0, rows, GCALL):
                        sn = min(GCALL, rows - s0)
                        nc.gpsimd.dma_gather(
                            out_ap=gl[:, s0 // 128:(s0 + sn) // 128, :],
                            in_ap=table[0:L_WIN, :],
                            idxs_ap=idxL_t[:, (roff + s0) // 16:(roff + s0 + sn) // 16],
                            num_idxs=sn, num_idxs_reg=sn, elem_size=D,
                            queue_num=next_q())
                    for b in range(bs, be):
                        o = int(128 * (off0[b] - off0[bs])) // 128
                        kb = int(k0[b])
                        nc.vector.tensor_reduce(
                            out=acc0[:, b, :],
                            in_=gl[:, o:o + kb, :].rearrange("p k d -> p d k"),
                            axis=mybir.AxisListType.X, op=mybir.AluOpType.add)

                # output stage per block
                for b in range(B):
                    tot = spool.tile([128, D], f32, tag="tot")
                    nc.vector.tensor_add(tot[:], acc0[:, b, :], accR[:, b, :])
                    scaled = spool.tile([128, D], f32, tag="scaled")
                    nc.scalar.activation(
                        out=scaled[:], in_=tot[:],
                        func=mybir.ActivationFunctionType.Copy,
                        scale=dinv_t[:, b:b + 1])
                    pT = ppool.tile([D, 128], f32, tag="pT")
                    nc.tensor.transpose(pT[:], scaled[:], ident[:])
                    accT = spool.tile([D, 128], f32, tag="accT")
                    nc.scalar.activation(
                        out=accT[:], in_=pT[:],
                        func=mybir.ActivationFunctionType.Copy)
                    pM = ppool.tile([Dl, 128], f32, tag="pM")
                    nc.tensor.matmul(pM[:], Wt[l][:], accT[:], start=True, stop=True)
                    hT = spool.tile([Dl, 128], f32, tag="hT")
                    if l < 2:
                        nc.scalar.activation(
                            out=hT[:], in_=pM[:],
                            func=mybir.ActivationFunctionType.Tanh,
                            bias=bt[l][:])
                    else:
                        nc.vector.tensor_scalar_add(hT[:], pM[:], bt[l][:])
                    p2 = ppool.tile([128, Dl], f32, tag="p2")
                    nc.tensor.transpose(p2[:], hT[:], ident[:Dl, :Dl])
                    res = spool.tile([128, Dl], f32, tag="res")
                    if l < 2:
                        nc.vector.tensor_scalar_mul(res[:], p2[:], dinv_t[:, b:b + 1])
                        nc.sync.dma_start(out=agins[l][b * 128:(b + 1) * 128, :], in_=res[:])
                    else:
                        nc.vector.tensor_copy(res[:], p2[:])
                        nc.sync.dma_start(out=z_ext[b * 128:(b + 1) * 128, :], in_=res[:])

                if l < 2:
                    nc.gpsimd.collective_compute(
                        "AllGather", mybir.AluOpType.bypass,
                        replica_groups=[list(range(C))],
                        ins=[agins[l][:]], outs=[tables[l + 1][:]])

    nc.finalize()
    return nc


def kernel(x, edge_index, W1, b1, W2, b2, W3, b3):
    global _last_results
    import os
    from concourse.bass_utils import run_bass_kernel_spmd

    meta, host = _prep(x, edge_index, W1, b1, W2, b2, W3, b3)
    nc = _build(meta)

    in_maps = []
    for c in range(C):
        in_maps.append({
            "x_table": host["x_table"],
            "idxL": host["idxL"][c], "idxH": host["idxH"][c],
            "idxR": host["idxR"][c], "dinv_t": host["dinv_t"][c],
            "W1": host["W1"], "W2": host["W2"], "W3": host["W3"],
            "b1": host["b1"], "b2": host["b2"], "b3": host["b3"],
        })
    res = run_bass_kernel_spmd(
        nc, in_maps, list(range(C)),
        trace=bool(int(os.environ.get("GCN_TRACE", "0"))))
    _last_results = res

    q0 = host["q0"]
    z = np.empty((N, DOUT), np.float32)
    for c in range(C):
        nodes = np.arange(c * NPC, (c + 1) * NPC)
        z[nodes] = res.results[c]["z"][q0[nodes]]
    return z



# revision 7
# speedup vs baseline: 1.3179x; 1.3179x over previous
"""3-layer GCN encoder on 8 Trainium2 NeuronCores.

Strategy (v2):
- Nodes sharded across 8 cores (6250 real nodes each, padded to 6272 = 49*128
  slots); edges partitioned by destination core.
- GCN norm is symmetric (norm = dinv[src]*dinv[dst]), so node rows in the
  gather table are pre-scaled by dinv; aggregation is a plain sum of gathered
  rows; the result is post-scaled by dinv[dst].
- Aggregate-then-transform: A_hat @ (h W) == (A_hat @ h) W, so every gather
  moves 64-dim (256B) rows regardless of layer.
- Scatter-add is eliminated on-device: destination nodes are degree-sorted
  into 128-node blocks; every node's edge list is padded to the block max
  degree k_b (pad slots point at an all-zero table row). Gather indices are
  laid out slot-major so edge j of block-node p lands at (partition p,
  chunk j) - segment-sum becomes a fixed-shape strided tensor_reduce per
  block.
- Self-loop edges are NOT gathered: the node's own table row is kept in SBUF
  from the previous layer's output stage and added directly.
- The replicated table is split by canonical position (total-degree sort):
  tableL = rows with pos < 3200 on each core (25600 rows), tableH = the rest
  (24576 rows). Both windows are < 32768 rows so int16 gather indices work.
- Each window's edge set is accumulated in its own window-degree-sorted
  destination order (tight block padding), then realigned to the canonical
  order with a 6272-row gather through a DRAM scratch buffer.
- Inter-layer exchange is two chunked AllGathers: blocks 25-48 of the output
  (the H chunk) are AllGathered as soon as they are computed; the next
  layer's H-window gathers depend only on AG-H, and the previous layer's
  AG-L trigger is issued in the middle of the next layer's H gathers so its
  input-wait never stalls the GpSimd queue.
"""

import numpy as np


def _install_ntff_hook_shim():
    """Pre-seed antenv.axon_hooks so trace=True can profile (no-op if the
    boot already registered the hook via a real antenv.axon_hooks)."""
    import sys
    import types

    if "antenv.axon_hooks" in sys.modules:
        return
    mod = types.ModuleType("antenv.axon_hooks")
    holder = [None]
    mod.set_axon_ntff_profile_hook = lambda h: holder.__setitem__(0, h)
    mod.get_axon_ntff_profile_hook = lambda: holder[0]
    sys.modules["antenv.axon_hooks"] = mod
    try:
        import antenv

        antenv.axon_hooks = mod
    except ImportError:
        pass


_install_ntff_hook_shim()

N = 50000
E = 800000
D = 64
DOUT = 32
C = 8
NPC = 6250            # real nodes per core
SLOTS = 6272          # padded slots per core = 49*128
B = 49                # dst blocks per core
BSPLIT = 25           # chunk boundary: blocks [0,25) = L chunk, [25,49) = H
POSL = BSPLIT * 128   # 3200 positions in the L chunk per core
POSH = SLOTS - POSL   # 3072 positions in the H chunk per core
LROWS = C * POSL      # 25600 rows in tableL
HROWS = C * POSH      # 24576 rows in tableH
HZERO = 6251 - POSL   # core-0 dummy row inside tableH (canonical pos 6251)
PIECE_CAP = 6144      # rows per gather buffer (whole blocks)
GCALL = 1024          # max rows per dma_gather call (SWDGE ring quantum)

_last_results = None  # BassKernelResults of the most recent run (for test.py)


def _wrap_idx(stream):
    """int32 stream [C, n] -> int16 [C, 128, n/16] wrapped+replicated tile."""
    n = stream.shape[-1]
    assert n % 16 == 0
    w = stream.reshape(-1, n // 16, 16)
    w = np.swapaxes(w, -1, -2)                     # [C, 16, n/16]
    w = np.tile(w, (1, 8, 1))
    return np.ascontiguousarray(w).astype(np.int16)


def _prep(x, edge_index, W1, b1, W2, b2, W3, b3):
    src = np.asarray(edge_index[0], dtype=np.int64)
    dst = np.asarray(edge_index[1], dtype=np.int64)

    deg = np.bincount(dst, minlength=N) + 1  # A + I degree on dst
    dinv = (1.0 / np.sqrt(deg.astype(np.float64))).astype(np.float32)

    # --- canonical per-core positions (total-degree sort) ---------------
    # pos 0 and pos 6251..6271 are dummy (all-zero) slots on every core.
    def core_sort(keys):
        q = np.empty(N, np.int64)
        for c in range(C):
            nodes = np.arange(c * NPC, (c + 1) * NPC)
            o = nodes[np.argsort(keys[nodes], kind="stable")]
            q[o] = 1 + np.arange(NPC)
        return q

    qC = core_sort(deg)
    core_of = np.arange(N) // NPC
    in_H = qC >= POSL
    # canonical window-relative source row of each node
    wrow = np.where(in_H, core_of * POSH + (qC - POSL), core_of * POSL + qC)

    src_H = in_H[src]
    dL = np.bincount(dst[~src_H], minlength=N)
    dH = np.bincount(dst[src_H], minlength=N)

    qLs = core_sort(dL)   # window-local accumulation orders
    qHs = core_sort(dH)

    # shared (across cores) per-block slot counts, in the window sort order
    def block_max(dw, qx):
        k = np.zeros(B, np.int64)
        for c in range(C):
            nodes = np.arange(c * NPC, (c + 1) * NPC)
            p = np.zeros(SLOTS, np.int64)
            p[qx[nodes]] = dw[nodes]
            k = np.maximum(k, p.reshape(B, 128).max(axis=1))
        return np.maximum(k, 1)

    kL = block_max(dL, qLs).astype(int)
    kH = block_max(dH, qHs).astype(int)
    offL = np.zeros(B + 1, np.int64)
    offL[1:] = np.cumsum(kL)
    offH = np.zeros(B + 1, np.int64)
    offH[1:] = np.cumsum(kH)
    L_len = int(128 * offL[-1])
    H_len = int(128 * offH[-1])

    # --- gather index streams -------------------------------------------
    def build_streams(sel, qx, offx, length, pad_row):
        es, ed = src[sel], dst[sel]
        core = ed // NPC
        qd = qx[ed]
        order = np.argsort(core * SLOTS + qd, kind="stable")
        es, core, qd = es[order], core[order], qd[order]
        key = core * SLOTS + qd
        starts = np.searchsorted(key, key)
        j = np.arange(len(key)) - starts
        b = qd // 128
        p = qd % 128
        pos = (offx[b] + j) * 128 + p
        streams = np.full((C, length), pad_row, np.int32)
        streams[core, pos] = wrow[es].astype(np.int32)
        return streams

    idxL = _wrap_idx(build_streams(~src_H, qLs, offL, L_len, 0))
    idxH = _wrap_idx(build_streams(src_H, qHs, offH, H_len, HZERO))

    # --- realign streams: canonical pos i -> scratch row of that node ----
    # scratch row of a node accumulated at window pos q = (q % 128) * B + q // 128
    def realign_stream(qx):
        r = np.empty((C, SLOTS), np.int32)
        for c in range(C):
            nodes = np.arange(c * NPC, (c + 1) * NPC)
            m = np.empty(SLOTS, np.int64)
            m[qC[nodes]] = qx[nodes]
            dummy_c = np.array([0] + list(range(NPC + 1, SLOTS)))
            m[dummy_c] = dummy_c  # dummies occupy the same pos in all sorts
            r[c] = ((m % 128) * B + m // 128).astype(np.int32)
        return _wrap_idx(r)

    idxRL = realign_stream(qLs)
    idxRH = realign_stream(qHs)

    # --- dinv tiles [C, 128, B] (canonical layout) ----------------------
    dinv_t = np.zeros((C, 128, B), np.float32)
    for c in range(C):
        nodes = np.arange(c * NPC, (c + 1) * NPC)
        arr = np.zeros(SLOTS, np.float32)
        arr[qC[nodes]] = dinv[nodes]
        dinv_t[c] = arr.reshape(B, 128).T

    # --- initial tables + own rows --------------------------------------
    x = np.asarray(x, np.float32)
    xs = x * dinv[:, None]
    xtL = np.zeros((LROWS, D), np.float32)
    xtH = np.zeros((HROWS, D), np.float32)
    xtL[wrow[~in_H]] = xs[~in_H]
    xtH[wrow[in_H]] = xs[in_H]
    own0 = np.zeros((C, 128, B * D), np.float32)
    for c in range(C):
        nodes = np.arange(c * NPC, (c + 1) * NPC)
        arr = np.zeros((SLOTS, D), np.float32)
        arr[qC[nodes]] = xs[nodes]
        own0[c] = arr.reshape(B, 128, D).transpose(1, 0, 2).reshape(128, B * D)

    meta = dict(kL=kL, kH=kH, offL=offL, offH=offH, L_len=L_len, H_len=H_len)
    host = dict(
        xtL=xtL, xtH=xtH, own0=own0,
        idxL=idxL, idxH=idxH, idxRL=idxRL, idxRH=idxRH, dinv_t=dinv_t,
        W1=np.asarray(W1, np.float32), W2=np.asarray(W2, np.float32),
        W3=np.asarray(W3, np.float32),
        b1=np.asarray(b1, np.float32).reshape(D, 1),
        b2=np.asarray(b2, np.float32).reshape(D, 1),
        b3=np.asarray(b3, np.float32).reshape(DOUT, 1),
        q0=qC,
    )
    return meta, host


def _pieces(kb, off, bs, be):
    """Group blocks [bs, be) into pieces with <= PIECE_CAP gathered rows."""
    out = []
    b0 = bs
    while b0 < be:
        b1 = b0
        rows = 0
        while b1 < be and (rows + 128 * kb[b1]) <= PIECE_CAP:
            rows += 128 * kb[b1]
            b1 += 1
        if b1 == b0:
            rows = 128 * kb[b0]
            b1 = b0 + 1
        out.append((b0, b1, int(128 * off[b0]), int(rows)))
        b0 = b1
    return out


def _build(meta):
    import concourse.bacc as bacc
    import concourse.mybir as mybir
    from concourse.tile import TileContext

    kL, kH = meta["kL"], meta["kH"]
    offL, offH = meta["offL"], meta["offH"]
    L_len, H_len = meta["L_len"], meta["H_len"]
    f32 = mybir.dt.float32
    i16 = mybir.dt.int16

    nc = bacc.Bacc(
        None,
        target_bir_lowering=False,
        num_swdge_queues=4,
        dynamic_dma_scratch_size=32768,
    )

    xtL = nc.declare_dram_parameter("xtL", [LROWS, D], f32, isOutput=False)
    xtH = nc.declare_dram_parameter("xtH", [HROWS, D], f32, isOutput=False)
    own0_p = nc.declare_dram_parameter("own0", [128, B * D], f32, isOutput=False)
    idxL_p = nc.declare_dram_parameter("idxL", [128, L_len // 16], i16, isOutput=False)
    idxH_p = nc.declare_dram_parameter("idxH", [128, H_len // 16], i16, isOutput=False)
    idxRL_p = nc.declare_dram_parameter("idxRL", [128, SLOTS // 16], i16, isOutput=False)
    idxRH_p = nc.declare_dram_parameter("idxRH", [128, SLOTS // 16], i16, isOutput=False)
    dinv_p = nc.declare_dram_parameter("dinv_t", [128, B], f32, isOutput=False)
    W_p = [nc.declare_dram_parameter(f"W{i+1}", [D, D if i < 2 else DOUT], f32, isOutput=False) for i in range(3)]
    b_p = [nc.declare_dram_parameter(f"b{i+1}", [D if i < 2 else DOUT, 1], f32, isOutput=False) for i in range(3)]
    z_ext = nc.declare_dram_parameter("z", [SLOTS, DOUT], f32, isOutput=True)

    tabL = [xtL]
    tabH = [xtH]
    aginsL = []
    aginsH = []
    for l in range(2):
        tabL.append(nc.dram_tensor(f"tableL{l+1}", [LROWS, D], f32, addr_space="Shared"))
        tabH.append(nc.dram_tensor(f"tableH{l+1}", [HROWS, D], f32, addr_space="Shared"))
        aginsL.append(nc.dram_tensor(f"aginsL{l}", [POSL, D], f32))
        aginsH.append(nc.dram_tensor(f"aginsH{l}", [POSH, D], f32))
    scr = {w: nc.dram_tensor(f"scratch{w}", [SLOTS, D], f32) for w in "LH"}

    pieces = {
        "H": _pieces(kH, offH, 0, B),
        "L": _pieces(kL, offL, 0, B),
    }
    qctr = [0]

    def next_q():
        q = qctr[0] % 4
        qctr[0] += 1
        return q

    with TileContext(nc) as tc:
        with (
            tc.tile_pool(name="const", bufs=1) as cpool,
            tc.tile_pool(name="acc", bufs=1) as apool,
            tc.tile_pool(name="gath", bufs=3) as gpool,
            tc.tile_pool(name="stage", bufs=4) as spool,
            tc.tile_pool(name="psum", bufs=2, space="PSUM") as ppool,
        ):
            # ---- persistent constants ----
            idx_t = {
                "L": cpool.tile([128, L_len // 16], i16, tag="idxL", name="idxLt"),
                "H": cpool.tile([128, H_len // 16], i16, tag="idxH", name="idxHt"),
            }
            idxR_t = {
                "L": cpool.tile([128, SLOTS // 16], i16, tag="idxRL", name="idxRLt"),
                "H": cpool.tile([128, SLOTS // 16], i16, tag="idxRH", name="idxRHt"),
            }
            dinv_t = cpool.tile([128, B], f32, tag="dinv")
            ident = cpool.tile([128, 128], f32, tag="ident")
            Wt = [cpool.tile([D, D if i < 2 else DOUT], f32, tag=f"W{i}", name=f"Wt{i}") for i in range(3)]
            bt = [cpool.tile([D if i < 2 else DOUT, 1], f32, tag=f"b{i}", name=f"bt{i}") for i in range(3)]
            own = [cpool.tile([128, B, D], f32, tag=f"own{i}", name=f"own{i}") for i in range(2)]

            nc.sync.dma_start(out=idx_t["L"][:], in_=idxL_p[:])
            nc.sync.dma_start(out=idx_t["H"][:], in_=idxH_p[:])
            nc.sync.dma_start(out=idxR_t["L"][:], in_=idxRL_p[:])
            nc.sync.dma_start(out=idxR_t["H"][:], in_=idxRH_p[:])
            nc.sync.dma_start(out=dinv_t[:], in_=dinv_p[:])
            nc.sync.dma_start(
                out=own[0][:].rearrange("p b d -> p (b d)"), in_=own0_p[:])
            for i in range(3):
                nc.sync.dma_start(out=Wt[i][:], in_=W_p[i][:])
                nc.sync.dma_start(out=bt[i][:], in_=b_p[i][:])
            nc.gpsimd.memset(ident[:], 1.0)
            nc.gpsimd.affine_select(
                out=ident[:], in_=ident[:], pattern=[[-1, 128]], base=0,
                channel_multiplier=1, compare_op=mybir.AluOpType.is_equal, fill=0.0)

            # ---- layers ----
            # AG-L of layer l-1 is issued in the middle of layer l (after the
            # H-window gathers, before the L-window gathers that consume it)
            # so its input-wait on the GpSimd queue never stalls gather issue.
            pending_agl = [None]

            for l in range(3):
                Dl = D if l < 2 else DOUT
                own_cur = own[l % 2]
                own_nxt = own[(l + 1) % 2]
                accs = {
                    "H": apool.tile([128, B, D], f32, tag="accH", name="accH"),
                    "L": apool.tile([128, B, D], f32, tag="accL", name="accL"),
                }
                accR = {
                    "H": apool.tile([128, B, D], f32, tag="accRH", name="accRH"),
                    "L": apool.tile([128, B, D], f32, tag="accRL", name="accRL"),
                }
                tabs = {"H": tabH[l], "L": tabL[l]}
                koffs = {"H": (kH, offH), "L": (kL, offL)}

                def run_pieces(win):
                    kb_arr, off_arr = koffs[win]
                    for (bs, be, roff, rows) in pieces[win]:
                        g = gpool.tile([128, rows // 128, D], f32, tag="g")
                        for s0 in range(0, rows, GCALL):
                            sn = min(GCALL, rows - s0)
                            nc.gpsimd.dma_gather(
                                out_ap=g[:, s0 // 128:(s0 + sn) // 128, :],
                                in_ap=tabs[win][:],
                                idxs_ap=idx_t[win][:, (roff + s0) // 16:(roff + s0 + sn) // 16],
                                num_idxs=sn, num_idxs_reg=sn, elem_size=D,
                                queue_num=next_q())
                        for b in range(bs, be):
                            o = int(off_arr[b] - off_arr[bs])
                            kb = int(kb_arr[b])
                            nc.vector.tensor_reduce(
                                out=accs[win][:, b, :],
                                in_=g[:, o:o + kb, :].rearrange("p k d -> p d k"),
                                axis=mybir.AxisListType.X, op=mybir.AluOpType.add)

                def realign(win):
                    nc.sync.dma_start(
                        out=scr[win][:].rearrange("(p b) d -> p b d", p=128),
                        in_=accs[win][:])
                    for s0 in range(0, SLOTS, GCALL):
                        sn = min(GCALL, SLOTS - s0)
                        nc.gpsimd.dma_gather(
                            out_ap=accR[win][:, s0 // 128:(s0 + sn) // 128, :],
                            in_ap=scr[win][:],
                            idxs_ap=idxR_t[win][:, s0 // 16:(s0 + sn) // 16],
                            num_idxs=sn, num_idxs_reg=sn, elem_size=D,
                            queue_num=next_q())

                def out_block(b):
                    tot = spool.tile([128, D], f32, tag="tot")
                    nc.vector.tensor_add(tot[:], accR["H"][:, b, :], accR["L"][:, b, :])
                    tot2 = spool.tile([128, D], f32, tag="tot2")
                    nc.vector.tensor_add(tot2[:], tot[:], own_cur[:, b, :])
                    scaled = spool.tile([128, D], f32, tag="scaled")
                    nc.scalar.activation(
                        out=scaled[:], in_=tot2[:],
                        func=mybir.ActivationFunctionType.Copy,
                        scale=dinv_t[:, b:b + 1])
                    pT = ppool.tile([D, 128], f32, tag="pT")
                    nc.tensor.transpose(pT[:], scaled[:], ident[:])
                    accT = spool.tile([D, 128], f32, tag="accT")
                    nc.scalar.activation(
                        out=accT[:], in_=pT[:],
                        func=mybir.ActivationFunctionType.Copy)
                    pM = ppool.tile([Dl, 128], f32, tag="pM")
                    nc.tensor.matmul(pM[:], Wt[l][:], accT[:], start=True, stop=True)
                    hT = spool.tile([Dl, 128], f32, tag="hT")
                    if l < 2:
                        nc.scalar.activation(
                            out=hT[:], in_=pM[:],
                            func=mybir.ActivationFunctionType.Tanh,
                            bias=bt[l][:])
                    else:
                        nc.vector.tensor_scalar_add(hT[:], pM[:], bt[l][:])
                    p2 = ppool.tile([128, Dl], f32, tag="p2")
                    nc.tensor.transpose(p2[:], hT[:], ident[:Dl, :Dl])
                    if l < 2:
                        nc.vector.tensor_scalar_mul(
                            own_nxt[:, b, :], p2[:], dinv_t[:, b:b + 1])
                        if b >= BSPLIT:
                            nc.sync.dma_start(
                                out=aginsH[l][(b - BSPLIT) * 128:(b - BSPLIT + 1) * 128, :],
                                in_=own_nxt[:, b, :])
                        else:
                            nc.sync.dma_start(
                                out=aginsL[l][b * 128:(b + 1) * 128, :],
                                in_=own_nxt[:, b, :])
                    else:
                        res = spool.tile([128, Dl], f32, tag="res")
                        nc.vector.tensor_copy(res[:], p2[:])
                        nc.sync.dma_start(
                            out=z_ext[b * 128:(b + 1) * 128, :], in_=res[:])

                # issue order chosen so in-order engine queues can overlap.
                run_pieces("H")
                if pending_agl[0] is not None:
                    lp = pending_agl[0]
                    nc.gpsimd.collective_compute(
                        "AllGather", mybir.AluOpType.bypass,
                        replica_groups=[list(range(C))],
                        ins=[aginsL[lp][:]], outs=[tabL[lp + 1][:]])
                    pending_agl[0] = None
                run_pieces("L")
                realign("H")
                realign("L")
                for b in range(BSPLIT, B):
                    out_block(b)
                if l < 2:
                    nc.gpsimd.collective_compute(
                        "AllGather", mybir.AluOpType.bypass,
                        replica_groups=[list(range(C))],
                        ins=[aginsH[l][:]], outs=[tabH[l + 1][:]])
                for b in range(BSPLIT):
                    out_block(b)
                if l < 2:
                    pending_agl[0] = l

    nc.finalize()
    return nc


def kernel(x, edge_index, W1, b1, W2, b2, W3, b3):
    global _last_results
    import os
    from concourse.bass_utils import run_bass_kernel_spmd

    meta, host = _prep(x, edge_index, W1, b1, W2, b2, W3, b3)
    nc = _build(meta)

    in_maps = []
    for c in range(C):
        in_maps.append({
            "xtL": host["xtL"], "xtH": host["xtH"], "own0": host["own0"][c],
            "idxL": host["idxL"][c], "idxH": host["idxH"][c],
            "idxRL": host["idxRL"][c], "idxRH": host["idxRH"][c],
            "dinv_t": host["dinv_t"][c],
            "W1": host["W1"], "W2": host["W2"], "W3": host["W3"],
            "b1": host["b1"], "b2": host["b2"], "b3": host["b3"],
        })
    res = run_bass_kernel_spmd(
        nc, in_maps, list(range(C)),
        trace=bool(int(os.environ.get("GCN_TRACE", "0"))))
    _last_results = res

    q0 = host["q0"]
    z = np.empty((N, DOUT), np.float32)
    for c in range(C):
        nodes = np.arange(c * NPC, (c + 1) * NPC)
        z[nodes] = res.results[c]["z"][q0[nodes]]
    return z


# revision 11
# speedup vs baseline: 1.4184x; 1.0763x over previous
"""3-layer GCN encoder on 8 Trainium2 NeuronCores.

Strategy (v3):
- Nodes sharded across 8 cores (6250 real nodes each, padded to 6272 = 49*128
  slots); edges partitioned by destination core.
- GCN norm is symmetric (norm = dinv[src]*dinv[dst]), so node rows in the
  gather table are pre-scaled by dinv; aggregation is a plain sum of gathered
  rows; the result is post-scaled by dinv[dst].
- Aggregate-then-transform: A_hat @ (h W) == (A_hat @ h) W, so every gather
  moves 64-dim (256B) rows regardless of layer.
- Scatter-add is eliminated on-device: destination nodes are sorted into
  128-node blocks; every node's edge list is padded to the block max degree
  k_b (pad slots point at an all-zero table row). Gather indices are laid
  out slot-major so edge j of block-node p lands at (partition p, chunk j) -
  segment-sum becomes a fixed-shape strided tensor_reduce per block.
- Self-loop edges are NOT gathered: the node's own table row is kept in SBUF
  from the previous layer's output stage and added directly.
- Chunk split: a per-core total-degree sort assigns each node to the lo
  chunk (3200 positions, blocks 0-24) or hi chunk (3072 positions, blocks
  25-48). The replicated table is two tensors: tableL = lo rows of all cores
  (25600 rows), tableH = hi rows (24576); both < 32768 so int16 gather
  indices work. The chunk is also the source window of every edge.
- The canonical order WITHIN each chunk sorts members by their L-window
  in-degree dL, so the L-window streams accumulate directly into canonical
  layout with tight block padding. The H window accumulates in a chunk-local
  dH-sorted order and is realigned per chunk with a small gather through a
  DRAM scratch buffer (hi realign only needs hi-block reduces, so the hi
  output stage starts early).
- Inter-layer exchange is two chunked AllGathers: AG-H fires after the hi
  blocks' outputs and overlaps the lo half of the layer; AG-L of layer l is
  triggered in the middle of layer l+1's H-window gathers so its input-wait
  never stalls the GpSimd queue.
"""

import numpy as np


def _install_ntff_hook_shim():
    """Pre-seed antenv.axon_hooks so trace=True can profile (no-op if the
    boot already registered the hook via a real antenv.axon_hooks)."""
    import sys
    import types

    if "antenv.axon_hooks" in sys.modules:
        return
    mod = types.ModuleType("antenv.axon_hooks")
    holder = [None]
    mod.set_axon_ntff_profile_hook = lambda h: holder.__setitem__(0, h)
    mod.get_axon_ntff_profile_hook = lambda: holder[0]
    sys.modules["antenv.axon_hooks"] = mod
    try:
        import antenv

        antenv.axon_hooks = mod
    except ImportError:
        pass


_install_ntff_hook_shim()

N = 50000
E = 800000
D = 64
DOUT = 32
C = 8
NPC = 6250            # real nodes per core
SLOTS = 6272          # padded slots per core = 49*128
B = 49                # dst blocks per core
BSPLIT = 25           # chunk boundary: blocks [0,25) = lo, [25,49) = hi
BHI = B - BSPLIT      # 24 hi blocks
POSL = BSPLIT * 128   # 3200 positions in the lo chunk per core
POSH = SLOTS - POSL   # 3072 positions in the hi chunk per core
LROWS = C * POSL      # 25600 rows in tableL
HROWS = C * POSH      # 24576 rows in tableH
HZERO = 6251 - POSL   # core-0 dummy row inside tableH (canonical pos 6251)
PIECE_CAP = 4096      # rows per gather buffer (whole blocks)
GCALL = 2048          # max rows per dma_gather call (half the SWDGE ring)

_last_results = None  # BassKernelResults of the most recent run (for test.py)


def _wrap_idx(stream):
    """int32 stream [C, n] -> int16 [C, 128, n/16] wrapped+replicated tile."""
    n = stream.shape[-1]
    assert n % 16 == 0
    w = stream.reshape(-1, n // 16, 16)
    w = np.swapaxes(w, -1, -2)                     # [C, 16, n/16]
    w = np.tile(w, (1, 8, 1))
    return np.ascontiguousarray(w).astype(np.int16)


def _prep(x, edge_index, W1, b1, W2, b2, W3, b3):
    src = np.asarray(edge_index[0], dtype=np.int64)
    dst = np.asarray(edge_index[1], dtype=np.int64)

    deg = np.bincount(dst, minlength=N) + 1  # A + I degree on dst
    dinv = (1.0 / np.sqrt(deg.astype(np.float64))).astype(np.float32)

    # --- chunk membership: per-core total-degree sort -------------------
    # Real positions 1..6250 in degree order; pos 0 and 6251..6271 dummy.
    memb_pos = np.empty(N, np.int64)
    for c in range(C):
        nodes = np.arange(c * NPC, (c + 1) * NPC)
        o = nodes[np.argsort(deg[nodes], kind="stable")]
        memb_pos[o] = 1 + np.arange(NPC)
    in_H = memb_pos >= POSL          # fixed chunk membership

    # window in-degrees (fixed once membership is fixed)
    src_H = in_H[src]
    dL = np.bincount(dst[~src_H], minlength=N)
    dH = np.bincount(dst[src_H], minlength=N)

    # --- canonical order: chunk-local sort by dL ------------------------
    # (so L-window accumulation is directly in canonical layout)
    def chunk_local_sort(keys):
        q = np.empty(N, np.int64)
        for c in range(C):
            nodes = np.arange(c * NPC, (c + 1) * NPC)
            lo = nodes[~in_H[nodes]]
            hi = nodes[in_H[nodes]]
            olo = lo[np.argsort(keys[lo], kind="stable")]
            ohi = hi[np.argsort(keys[hi], kind="stable")]
            q[olo] = 1 + np.arange(len(olo))            # pos 1..3199
            q[ohi] = POSL + np.arange(len(ohi))         # pos 3200..6250
        return q

    qC = chunk_local_sort(dL)
    qHs = chunk_local_sort(dH)   # H-window accumulation order

    core_of = np.arange(N) // NPC
    # canonical window-relative source row of each node
    wrow = np.where(in_H, core_of * POSH + (qC - POSL), core_of * POSL + qC)

    # shared (across cores) per-block slot counts
    def block_max(dw, qx):
        k = np.zeros(B, np.int64)
        for c in range(C):
            nodes = np.arange(c * NPC, (c + 1) * NPC)
            p = np.zeros(SLOTS, np.int64)
            p[qx[nodes]] = dw[nodes]
            k = np.maximum(k, p.reshape(B, 128).max(axis=1))
        return np.maximum(k, 1)

    kL = block_max(dL, qC).astype(int)
    kH = block_max(dH, qHs).astype(int)
    offL = np.zeros(B + 1, np.int64)
    offL[1:] = np.cumsum(kL)
    offH = np.zeros(B + 1, np.int64)
    offH[1:] = np.cumsum(kH)
    L_len = int(128 * offL[-1])
    H_len = int(128 * offH[-1])

    # --- gather index streams -------------------------------------------
    def build_streams(sel, qx, offx, length, pad_row):
        es, ed = src[sel], dst[sel]
        core = ed // NPC
        qd = qx[ed]
        order = np.argsort(core * SLOTS + qd, kind="stable")
        es, core, qd = es[order], core[order], qd[order]
        key = core * SLOTS + qd
        starts = np.searchsorted(key, key)
        j = np.arange(len(key)) - starts
        b = qd // 128
        p = qd % 128
        pos = (offx[b] + j) * 128 + p
        streams = np.full((C, length), pad_row, np.int32)
        streams[core, pos] = wrow[es].astype(np.int32)
        return streams

    idxL = _wrap_idx(build_streams(~src_H, qC, offL, L_len, 0))
    idxH = _wrap_idx(build_streams(src_H, qHs, offH, H_len, HZERO))

    # --- H realign streams (per chunk) ----------------------------------
    # scratch row of a node accumulated at H-order pos q:
    #   lo chunk: (q % 128) * BSPLIT + q // 128
    #   hi chunk: (q % 128) * BHI + (q // 128 - BSPLIT)
    mq = np.empty((C, SLOTS), np.int64)   # canonical pos -> H-order pos
    for c in range(C):
        nodes = np.arange(c * NPC, (c + 1) * NPC)
        mq[c][qC[nodes]] = qHs[nodes]
        dummy_c = np.array([0] + list(range(NPC + 1, SLOTS)))
        mq[c][dummy_c] = dummy_c
    mlo = mq[:, :POSL]
    mhi = mq[:, POSL:]
    idxRH_lo = _wrap_idx(((mlo % 128) * BSPLIT + mlo // 128).astype(np.int32))
    idxRH_hi = _wrap_idx(
        ((mhi % 128) * BHI + (mhi // 128 - BSPLIT)).astype(np.int32))

    # --- dinv tiles [C, 128, B] (canonical layout) ----------------------
    dinv_t = np.zeros((C, 128, B), np.float32)
    for c in range(C):
        nodes = np.arange(c * NPC, (c + 1) * NPC)
        arr = np.zeros(SLOTS, np.float32)
        arr[qC[nodes]] = dinv[nodes]
        dinv_t[c] = arr.reshape(B, 128).T

    # --- initial tables + own rows --------------------------------------
    x = np.asarray(x, np.float32)
    xs = x * dinv[:, None]
    xtL = np.zeros((LROWS, D), np.float32)
    xtH = np.zeros((HROWS, D), np.float32)
    xtL[wrow[~in_H]] = xs[~in_H]
    xtH[wrow[in_H]] = xs[in_H]
    own0 = np.zeros((C, 128, B * D), np.float32)
    for c in range(C):
        nodes = np.arange(c * NPC, (c + 1) * NPC)
        arr = np.zeros((SLOTS, D), np.float32)
        arr[qC[nodes]] = xs[nodes]
        own0[c] = arr.reshape(B, 128, D).transpose(1, 0, 2).reshape(128, B * D)

    meta = dict(kL=kL, kH=kH, offL=offL, offH=offH, L_len=L_len, H_len=H_len)
    host = dict(
        xtL=xtL, xtH=xtH, own0=own0,
        idxL=idxL, idxH=idxH, idxRH_lo=idxRH_lo, idxRH_hi=idxRH_hi,
        dinv_t=dinv_t,
        W1=np.asarray(W1, np.float32), W2=np.asarray(W2, np.float32),
        W3=np.asarray(W3, np.float32),
        b1=np.asarray(b1, np.float32).reshape(D, 1),
        b2=np.asarray(b2, np.float32).reshape(D, 1),
        b3=np.asarray(b3, np.float32).reshape(DOUT, 1),
        q0=qC,
    )
    return meta, host


def _pieces(kb, off, bs, be):
    """Group blocks [bs, be) into pieces with <= PIECE_CAP gathered rows."""
    out = []
    b0 = bs
    while b0 < be:
        b1 = b0
        rows = 0
        while b1 < be and (rows + 128 * kb[b1]) <= PIECE_CAP:
            rows += 128 * kb[b1]
            b1 += 1
        if b1 == b0:
            rows = 128 * kb[b0]
            b1 = b0 + 1
        out.append((b0, b1, int(128 * off[b0]), int(rows)))
        b0 = b1
    return out


def _build(meta):
    import concourse.bacc as bacc
    import concourse.mybir as mybir
    from concourse.tile import TileContext

    kL, kH = meta["kL"], meta["kH"]
    offL, offH = meta["offL"], meta["offH"]
    L_len, H_len = meta["L_len"], meta["H_len"]
    f32 = mybir.dt.float32
    i16 = mybir.dt.int16

    nc = bacc.Bacc(
        None,
        target_bir_lowering=False,
        num_swdge_queues=4,
        dynamic_dma_scratch_size=65536,
    )

    xtL = nc.declare_dram_parameter("xtL", [LROWS, D], f32, isOutput=False)
    xtH = nc.declare_dram_parameter("xtH", [HROWS, D], f32, isOutput=False)
    own0_p = nc.declare_dram_parameter("own0", [128, B * D], f32, isOutput=False)
    idxL_p = nc.declare_dram_parameter("idxL", [128, L_len // 16], i16, isOutput=False)
    idxH_p = nc.declare_dram_parameter("idxH", [128, H_len // 16], i16, isOutput=False)
    idxRHlo_p = nc.declare_dram_parameter("idxRH_lo", [128, POSL // 16], i16, isOutput=False)
    idxRHhi_p = nc.declare_dram_parameter("idxRH_hi", [128, POSH // 16], i16, isOutput=False)
    dinv_p = nc.declare_dram_parameter("dinv_t", [128, B], f32, isOutput=False)
    W_p = [nc.declare_dram_parameter(f"W{i+1}", [D, D if i < 2 else DOUT], f32, isOutput=False) for i in range(3)]
    b_p = [nc.declare_dram_parameter(f"b{i+1}", [D if i < 2 else DOUT, 1], f32, isOutput=False) for i in range(3)]
    z_ext = nc.declare_dram_parameter("z", [SLOTS, DOUT], f32, isOutput=True)

    tabL = [xtL]
    tabH = [xtH]
    aginsL = []
    aginsH = []
    for l in range(2):
        tabL.append(nc.dram_tensor(f"tableL{l+1}", [LROWS, D], f32, addr_space="Shared"))
        tabH.append(nc.dram_tensor(f"tableH{l+1}", [HROWS, D], f32, addr_space="Shared"))
        aginsL.append(nc.dram_tensor(f"aginsL{l}", [POSL, D], f32))
        aginsH.append(nc.dram_tensor(f"aginsH{l}", [POSH, D], f32))
    scr_lo = nc.dram_tensor("scrHlo", [POSL, D], f32)
    scr_hi = nc.dram_tensor("scrHhi", [POSH, D], f32)

    pieces = {
        ("H", "hi"): _pieces(kH, offH, BSPLIT, B),
        ("H", "lo"): _pieces(kH, offH, 0, BSPLIT),
        ("L", "hi"): _pieces(kL, offL, BSPLIT, B),
        ("L", "lo"): _pieces(kL, offL, 0, BSPLIT),
    }
    qctr = [0]

    def next_q():
        q = qctr[0] % 4
        qctr[0] += 1
        return q

    with TileContext(nc) as tc:
        with (
            tc.tile_pool(name="const", bufs=1) as cpool,
            tc.tile_pool(name="acc", bufs=1) as apool,
            tc.tile_pool(name="gath", bufs=3) as gpool,
            tc.tile_pool(name="stage", bufs=4) as spool,
            tc.tile_pool(name="psum", bufs=2, space="PSUM") as ppool,
        ):
            # ---- persistent constants ----
            idx_t = {
                "L": cpool.tile([128, L_len // 16], i16, tag="idxL", name="idxLt"),
                "H": cpool.tile([128, H_len // 16], i16, tag="idxH", name="idxHt"),
            }
            idxR_t = {
                "lo": cpool.tile([128, POSL // 16], i16, tag="idxRlo", name="idxRlo"),
                "hi": cpool.tile([128, POSH // 16], i16, tag="idxRhi", name="idxRhi"),
            }
            dinv_t = cpool.tile([128, B], f32, tag="dinv")
            ident = cpool.tile([128, 128], f32, tag="ident")
            Wt = [cpool.tile([D, D if i < 2 else DOUT], f32, tag=f"W{i}", name=f"Wt{i}") for i in range(3)]
            bt = [cpool.tile([D if i < 2 else DOUT, 1], f32, tag=f"b{i}", name=f"bt{i}") for i in range(3)]
            own = [cpool.tile([128, B, D], f32, tag=f"own{i}", name=f"own{i}") for i in range(2)]

            nc.sync.dma_start(out=idx_t["H"][:], in_=idxH_p[:])
            nc.sync.dma_start(out=idx_t["L"][:], in_=idxL_p[:])
            nc.sync.dma_start(out=idxR_t["hi"][:], in_=idxRHhi_p[:])
            nc.sync.dma_start(out=idxR_t["lo"][:], in_=idxRHlo_p[:])
            nc.sync.dma_start(out=dinv_t[:], in_=dinv_p[:])
            for i in range(3):
                nc.sync.dma_start(out=Wt[i][:], in_=W_p[i][:])
                nc.sync.dma_start(out=bt[i][:], in_=b_p[i][:])
            nc.sync.dma_start(
                out=own[0][:].rearrange("p b d -> p (b d)"), in_=own0_p[:])
            nc.gpsimd.memset(ident[:], 1.0)
            nc.gpsimd.affine_select(
                out=ident[:], in_=ident[:], pattern=[[-1, 128]], base=0,
                channel_multiplier=1, compare_op=mybir.AluOpType.is_equal, fill=0.0)

            # ---- layers ----
            pending_agl = [None]

            for l in range(3):
                Dl = D if l < 2 else DOUT
                own_cur = own[l % 2]
                own_nxt = own[(l + 1) % 2]
                accL = apool.tile([128, B, D], f32, tag="accL", name="accL")
                accH = apool.tile([128, B, D], f32, tag="accH", name="accH")
                accRH = apool.tile([128, B, D], f32, tag="accRH", name="accRH")
                tabs = {"H": tabH[l], "L": tabL[l]}
                accs = {"H": accH, "L": accL}
                koffs = {"H": (kH, offH), "L": (kL, offL)}

                def run_pieces(win, chunk, after_piece=None):
                    kb_arr, off_arr = koffs[win]
                    for pi, (bs, be, roff, rows) in enumerate(pieces[(win, chunk)]):
                        g = gpool.tile([128, rows // 128, D], f32, tag="g")
                        for s0 in range(0, rows, GCALL):
                            sn = min(GCALL, rows - s0)
                            nc.gpsimd.dma_gather(
                                out_ap=g[:, s0 // 128:(s0 + sn) // 128, :],
                                in_ap=tabs[win][:],
                                idxs_ap=idx_t[win][:, (roff + s0) // 16:(roff + s0 + sn) // 16],
                                num_idxs=sn, num_idxs_reg=sn, elem_size=D,
                                queue_num=next_q())
                        for b in range(bs, be):
                            o = int(off_arr[b] - off_arr[bs])
                            kb = int(kb_arr[b])
                            nc.vector.tensor_reduce(
                                out=accs[win][:, b, :],
                                in_=g[:, o:o + kb, :].rearrange("p k d -> p d k"),
                                axis=mybir.AxisListType.X, op=mybir.AluOpType.add)
                        if after_piece is not None and pi == after_piece[0]:
                            after_piece[1]()

                def realign(chunk):
                    if chunk == "hi":
                        scr, nb, b0 = scr_hi, BHI, BSPLIT
                    else:
                        scr, nb, b0 = scr_lo, BSPLIT, 0
                    nc.sync.dma_start(
                        out=scr[:].rearrange("(p b) d -> p b d", p=128),
                        in_=accH[:, b0:b0 + nb, :])
                    npos = nb * 128
                    for s0 in range(0, npos, GCALL):
                        sn = min(GCALL, npos - s0)
                        nc.gpsimd.dma_gather(
                            out_ap=accRH[:, b0 + s0 // 128:b0 + (s0 + sn) // 128, :],
                            in_ap=scr[:],
                            idxs_ap=idxR_t[chunk][:, s0 // 16:(s0 + sn) // 16],
                            num_idxs=sn, num_idxs_reg=sn, elem_size=D,
                            queue_num=next_q())

                def out_block(b):
                    tot = spool.tile([128, D], f32, tag="tot")
                    nc.vector.tensor_add(tot[:], accL[:, b, :], accRH[:, b, :])
                    tot2 = spool.tile([128, D], f32, tag="tot2")
                    nc.vector.tensor_add(tot2[:], tot[:], own_cur[:, b, :])
                    scaled = spool.tile([128, D], f32, tag="scaled")
                    nc.scalar.activation(
                        out=scaled[:], in_=tot2[:],
                        func=mybir.ActivationFunctionType.Copy,
                        scale=dinv_t[:, b:b + 1])
                    pT = ppool.tile([D, 128], f32, tag="pT")
                    nc.tensor.transpose(pT[:], scaled[:], ident[:])
                    accT = spool.tile([D, 128], f32, tag="accT")
                    nc.scalar.activation(
                        out=accT[:], in_=pT[:],
                        func=mybir.ActivationFunctionType.Copy)
                    pM = ppool.tile([Dl, 128], f32, tag="pM")
                    nc.tensor.matmul(pM[:], Wt[l][:], accT[:], start=True, stop=True)
                    hT = spool.tile([Dl, 128], f32, tag="hT")
                    if l < 2:
                        nc.scalar.activation(
                            out=hT[:], in_=pM[:],
                            func=mybir.ActivationFunctionType.Tanh,
                            bias=bt[l][:])
                    else:
                        nc.vector.tensor_scalar_add(hT[:], pM[:], bt[l][:])
                    p2 = ppool.tile([128, Dl], f32, tag="p2")
                    nc.tensor.transpose(p2[:], hT[:], ident[:Dl, :Dl])
                    if l < 2:
                        nc.vector.tensor_scalar_mul(
                            own_nxt[:, b, :], p2[:], dinv_t[:, b:b + 1])
                        if b >= BSPLIT:
                            nc.sync.dma_start(
                                out=aginsH[l][(b - BSPLIT) * 128:(b - BSPLIT + 1) * 128, :],
                                in_=own_nxt[:, b, :])
                        else:
                            nc.sync.dma_start(
                                out=aginsL[l][b * 128:(b + 1) * 128, :],
                                in_=own_nxt[:, b, :])
                    else:
                        res = spool.tile([128, Dl], f32, tag="res")
                        nc.vector.tensor_copy(res[:], p2[:])
                        nc.sync.dma_start(
                            out=z_ext[b * 128:(b + 1) * 128, :], in_=res[:])

                # issue order chosen so in-order engine queues can overlap.
                # AG-L of the previous layer is triggered after the FIRST H
                # piece (its input is long since ready, so the trigger clears
                # instantly and the collective runs during the H phase).
                def trigger_agl():
                    if pending_agl[0] is not None:
                        lp = pending_agl[0]
                        nc.gpsimd.collective_compute(
                            "AllGather", mybir.AluOpType.bypass,
                            replica_groups=[list(range(C))],
                            ins=[aginsL[lp][:]], outs=[tabL[lp + 1][:]])
                        pending_agl[0] = None

                def trigger_agh():
                    nc.gpsimd.collective_compute(
                        "AllGather", mybir.AluOpType.bypass,
                        replica_groups=[list(range(C))],
                        ins=[aginsH[l][:]], outs=[tabH[l + 1][:]])

                if l < 2:
                    run_pieces("H", "hi", after_piece=(0, trigger_agl))
                    run_pieces("H", "lo")
                    trigger_agl()  # no-op if already fired
                    run_pieces("L", "hi")
                    realign("hi")
                    for b in range(BSPLIT, B):
                        out_block(b)
                    run_pieces("L", "lo", after_piece=(1, trigger_agh))
                    realign("lo")
                    for b in range(BSPLIT):
                        out_block(b)
                    pending_agl[0] = l
                else:
                    # last layer: no AllGathers; finish the lo chunk first so
                    # the tail is the smaller hi chunk's output stage.
                    run_pieces("H", "lo", after_piece=(0, trigger_agl))
                    run_pieces("H", "hi")
                    trigger_agl()
                    run_pieces("L", "lo")
                    realign("lo")
                    for b in range(BSPLIT):
                        out_block(b)
                    run_pieces("L", "hi")
                    realign("hi")
                    for b in range(BSPLIT, B):
                        out_block(b)

    nc.finalize()
    return nc


def kernel(x, edge_index, W1, b1, W2, b2, W3, b3):
    global _last_results
    import os
    from concourse.bass_utils import run_bass_kernel_spmd

    meta, host = _prep(x, edge_index, W1, b1, W2, b2, W3, b3)
    nc = _build(meta)

    in_maps = []
    for c in range(C):
        in_maps.append({
            "xtL": host["xtL"], "xtH": host["xtH"], "own0": host["own0"][c],
            "idxL": host["idxL"][c], "idxH": host["idxH"][c],
            "idxRH_lo": host["idxRH_lo"][c], "idxRH_hi": host["idxRH_hi"][c],
            "dinv_t": host["dinv_t"][c],
            "W1": host["W1"], "W2": host["W2"], "W3": host["W3"],
            "b1": host["b1"], "b2": host["b2"], "b3": host["b3"],
        })
    res = run_bass_kernel_spmd(
        nc, in_maps, list(range(C)),
        trace=bool(int(os.environ.get("GCN_TRACE", "0"))))
    _last_results = res

    q0 = host["q0"]
    z = np.empty((N, DOUT), np.float32)
    for c in range(C):
        nodes = np.arange(c * NPC, (c + 1) * NPC)
        z[nodes] = res.results[c]["z"][q0[nodes]]
    return z
